# revision 7
# baseline (speedup 1.0000x reference)
"""Trainium2 Bass kernel for nn_Block_27384711479862 (metaformer block), v2.

Data parallel over batch B=8 -> 8 cores. Per-core x: [C=384, N=2304].

Strategy (fp8 e4m3 + DoubleRow matmuls):
  - q/k projections fused on host: u' = (kw'^T qw'/sqrt(C)) x + w2, so
    scores come from one projection: sT[m,n] = x_m . u'_n.  Computing the
    TRANSPOSED scores means exp(sT) is directly the rhs of the attention
    matmul - the N^2 PE transpose pass of the usual layout disappears.
  - softmax denominator l[n] = sum_m exp(sT[m,n]) falls out of the
    attention matmul itself: a ones-column is appended to vT (4th channel
    block), so l arrives as psum partition row 0.  1/l (per query n) is
    applied with a DMA-broadcast row (recip -> DRAM -> stride-0 broadcast).
  - all big matmuls are fp8 DoubleRow (2 x 128-row contraction chunks per
    instruction, 0.5 cycles/output column): scores, attention, qkv/proj,
    fc1/fc2.  Contractions over C=384 are zero-padded to 512 (2 DR pairs).
  - depthwise 3x3 runs on the PE as 5 DoubleRow tap-pair diag matmuls over
    a 50x50 zero-padded fp8 image (tap pairs share one shifted 4-dim AP).
  - layer-scale (ls1=ls2=0.01) damps branch errors 100x, so fp8 noise in
    the branches is far below the 2e-2 gate; the trunk (x, x1, out) stays
    f32 end to end.
"""
import numpy as np
import ml_dtypes

C = 384
HID = 1536
H = W = 48
N = H * W               # 2304
NC = 3                  # C chunks of 128
NCP = 4                 # zero-padded chunks (DoubleRow pairs)
NH = 12                 # HID chunks of 128
NRB = 18                # key blocks of 128
EPS = 1e-5
FP8 = ml_dtypes.float8_e4m3
FMAX = 224.0            # trn fp8e4 saturates at 240; keep margin
EXPB = -3.0             # exp margin (cancels in softmax)
RPT = 8                 # spatial rows per MLP tile
PAD = 50
PADN = PAD * PAD + 4    # +4 slack so the zero-tap pair read stays in bounds
NT5 = [(i * 512, min(512, N - i * 512)) for i in range((N + 511) // 512)]
NT6 = [(i * 384, 384) for i in range(6)]
GROUPS = [(0, 512), (512, 512), (1024, 512), (1536, 512), (2048, 256)]
PAIRS = [(0, 1024, GROUPS[0:2]), (1024, 1024, GROUPS[2:4]),
         (2048, 256, GROUPS[4:5])]
# dw tap-pair base offsets (relative to y0*PAD) and in-pair strides;
# taps (dy,dx) row-major 0..8, slot 9 = zero pad
DWPAIRS = [(0, 1), (2, 48), (51, 1), (100, 1), (102, 1)]
PW = 1024               # scores pair width

_PROG = None


def _build_program(iters=1):
    import concourse.bacc as bacc
    import concourse.bass as bass
    import concourse.mybir as mybir
    import concourse.tile as tile
    from contextlib import ExitStack

    dt = mybir.dt
    AF = mybir.ActivationFunctionType
    ALU = mybir.AluOpType
    PM = mybir.MatmulPerfMode
    DR = PM.DoubleRow
    f32, fp8 = dt.float32, dt.float8e4

    nc = bacc.Bacc("TRN2", target_bir_lowering=False, debug=False,
                   enable_asserts=False)

    def din(name, shape, d=f32):
        return nc.dram_tensor(name, list(shape), d, kind="ExternalInput").ap()

    x_d0 = din("x", (C, N))
    A8_d = din("A8", (128, NCP * C), fp8)
    vw8_d = din("vw8", (128, NCP * C), fp8)
    pw8_d = din("pw8", (128, NCP * C), fp8)
    f1w8_d = din("f1w8", (128, NCP * HID), fp8)
    f2w8_d = din("f2w8", (128, NH * C), fp8)
    dwd8_d = din("dwd8", (128, NH * 10 * 128), fp8)
    cc_d = din("cc", (128, 48))
    out_d0 = nc.dram_tensor("out", [C, N], f32, kind="ExternalOutput").ap()
    chain = [x_d0]
    for i in range(1, iters):
        chain.append(nc.dram_tensor(f"mid{i}", [C, N], f32).ap())
    chain.append(out_d0)
    rd_ds = [nc.dram_tensor(f"rd{i}", [1, N], f32).ap() for i in range(iters)]

    # cc column indices
    CAX, CSU, CW2, CSX, CEB, CSV, CSAT, CSP = 0, 1, 2, 5, 6, 7, 8, 9
    CB1, CA1, CAX1, CSH, CHB, CSG, CDWB, CSF2 = 10, 13, 16, 17, 18, 30, 31, 43

    with tile.TileContext(nc) as tc, ExitStack() as glob:
        consts = glob.enter_context(tc.tile_pool(name="consts", bufs=1))
        cc_s = consts.tile([128, 48], f32, tag="cc", name="cc")
        nc.sync.dma_start(cc_s[:], cc_d)
        A8_s = consts.tile([128, NCP * C], fp8, tag="A8", name="A8")
        nc.sync.dma_start(A8_s[:], A8_d)
        vw8_s = consts.tile([128, NCP * C], fp8, tag="vw8", name="vw8")
        nc.sync.dma_start(vw8_s[:], vw8_d)
        pw8_s = consts.tile([128, NCP * C], fp8, tag="pw8", name="pw8")
        nc.sync.dma_start(pw8_s[:], pw8_d)
        f1w8_s = consts.tile([128, NCP * HID], fp8, tag="f1w8", name="f1w8")
        nc.sync.dma_start(f1w8_s[:], f1w8_d)
        f2w8_s = consts.tile([128, NH * C], fp8, tag="f2w8", name="f2w8")
        nc.sync.dma_start(f2w8_s[:], f2w8_d)
        dwd8_s = consts.tile([128, NH * 10 * 128], fp8, tag="dwd8",
                             name="dwd8")
        nc.sync.dma_start(dwd8_s[:], dwd8_d)

        A8v = A8_s[:].rearrange("p (k f) -> p k f", k=NCP)
        vw8v = vw8_s[:].rearrange("p (k f) -> p k f", k=NCP)
        pw8v = pw8_s[:].rearrange("p (k f) -> p k f", k=NCP)
        f1w8v = f1w8_s[:].rearrange("p (k f) -> p k f", k=NCP)
        f2w8v = f2w8_s[:].rearrange("p (k f) -> p k f", k=NH)

        for it in range(iters):
          x_d, out_d, rd_d = chain[it], chain[it + 1], rd_ds[it]
          with ExitStack() as top:
            pmm = top.enter_context(
                tc.tile_pool(name="pmm", bufs=2, space="PSUM"))
            x1p = top.enter_context(tc.tile_pool(name="x1p", bufs=1))

            x1_t = [x1p.tile([128, N], f32, tag=f"x1_{c}", name=f"x1_{c}")
                    for c in range(NC)]
            x18_t = x1p.tile([128, NCP * N], fp8, tag="x18", name="x18")
            x18v = x18_t[:].rearrange("p (k f) -> p k f", k=NCP)
            nc.gpsimd.memset(x18v[:, NC, :], 0.0)

            with ExitStack() as attn_scope:
              ap8 = attn_scope.enter_context(tc.tile_pool(name="ap8", bufs=1))
              aTp = attn_scope.enter_context(tc.tile_pool(name="aTp", bufs=2))
              stp = attn_scope.enter_context(tc.tile_pool(name="stp", bufs=4))
              rbp = attn_scope.enter_context(tc.tile_pool(name="rbp", bufs=2))
              psc = attn_scope.enter_context(
                  tc.tile_pool(name="psc", bufs=2, space="PSUM"))
              pat = attn_scope.enter_context(
                  tc.tile_pool(name="pat", bufs=2, space="PSUM"))

              x8_t = ap8.tile([128, NCP * N], fp8, tag="x8", name="x8")
              x8v = x8_t[:].rearrange("p (k f) -> p k f", k=NCP)
              u8_t = ap8.tile([128, NCP * N], fp8, tag="u8", name="u8")
              u8v = u8_t[:].rearrange("p (k f) -> p k f", k=NCP)
              vT8_t = ap8.tile([128, NRB * 512], fp8, tag="vT8", name="vT8")
              vT8v = vT8_t[:].rearrange("p (k f) -> p k f", k=NRB)
              at8_t = ap8.tile([128, NCP * N], fp8, tag="at8", name="at8")
              at8v = at8_t[:].rearrange("p (k f) -> p k f", k=NCP)
              nc.gpsimd.memset(x8v[:, NC, :], 0.0)
              nc.gpsimd.memset(u8v[:, NC, :], 0.0)
              nc.gpsimd.memset(at8v[:, NC, :], 0.0)
              nc.gpsimd.memset(vT8v[:, :, 385:512], 0.0)
              nc.gpsimd.memset(vT8v[:, :, 384:385], 1.0)

              with ExitStack() as xload:
                xp = xload.enter_context(tc.tile_pool(name="xp", bufs=1))
                x_t = [xp.tile([128, N], f32, tag=f"x_{c}", name=f"x_{c}")
                       for c in range(NC)]
                # tiled loads + conversions so downstream PE work can
                # start before the whole image has landed
                for (n0, nn) in NT5:
                  for c in range(NC):
                    nc.sync.dma_start(x_t[c][:, n0:n0 + nn],
                                      x_d[c * 128:(c + 1) * 128, n0:n0 + nn])
                  for c in range(NC):
                    nc.gpsimd.tensor_scalar(
                        out=x8v[:, c, n0:n0 + nn], in0=x_t[c][:, n0:n0 + nn],
                        scalar1=cc_s[:, CAX:CAX + 1], scalar2=None,
                        op0=ALU.mult)
                    # x1pre = alpha1 * x + (beta1 + f2b)
                    nc.gpsimd.tensor_scalar(
                        out=x1_t[c][:, n0:n0 + nn], in0=x_t[c][:, n0:n0 + nn],
                        scalar1=cc_s[:, CA1 + c:CA1 + c + 1],
                        scalar2=cc_s[:, CB1 + c:CB1 + c + 1],
                        op0=ALU.mult, op1=ALU.add)

                # ---- u' = A x + w2 (DR), u8 = au*u' --------------------
                for (n0, nn) in NT5:
                  for mc in range(NC):
                    ps = pmm.tile([128, 512], f32, tag="mm", name="mm")
                    for k in range(2):
                      nc.tensor.matmul(
                          ps[:, :nn],
                          A8v[:, 2 * k:2 * k + 2, mc * 128:(mc + 1) * 128],
                          x8v[:, 2 * k:2 * k + 2, n0:n0 + nn],
                          start=(k == 0), stop=(k == 1), perf_mode=DR)
                    nc.vector.tensor_scalar(
                        out=u8v[:, mc, n0:n0 + nn], in0=ps[:, :nn],
                        scalar1=cc_s[:, CSU:CSU + 1],
                        scalar2=cc_s[:, CW2 + mc:CW2 + mc + 1],
                        op0=ALU.mult, op1=ALU.add)

                # ---- vT[m, c] (DR), vT8 = av*vT ------------------------
                for nb in range(NRB):
                  ps = pmm.tile([128, 512], f32, tag="mm", name="mm")
                  for k in range(2):
                    nc.tensor.matmul(
                        ps[:, :C],
                        x8v[:, 2 * k:2 * k + 2, nb * 128:(nb + 1) * 128],
                        vw8v[:, 2 * k:2 * k + 2, :],
                        start=(k == 0), stop=(k == 1), perf_mode=DR)
                  nc.vector.tensor_scalar(
                      out=vT8_t[:, nb * 512:nb * 512 + C], in0=ps[:, :C],
                      scalar1=cc_s[:, CSV:CSV + 1], scalar2=None,
                      op0=ALU.mult)

              # ---- attention, pair-of-groups at a time -----------------
              for (p0, pw_, grps) in PAIRS:
                aT8_t = aTp.tile([128, NRB * PW], fp8, tag="aT8", name="aT8")
                aT8v = aT8_t[:].rearrange("p (k f) -> p k f", k=NRB)
                for mb in range(NRB):
                  ps = psc.tile([128, PW], f32, tag="sc", name="sc")
                  for h0 in range(0, pw_, 512):
                    hw = min(512, pw_ - h0)
                    for k in range(2):
                      nc.tensor.matmul(
                          ps[:, h0:h0 + hw],
                          x8v[:, 2 * k:2 * k + 2, mb * 128:(mb + 1) * 128],
                          u8v[:, 2 * k:2 * k + 2, p0 + h0:p0 + h0 + hw],
                          start=(k == 0), stop=(k == 1), perf_mode=DR)
                  nc.scalar.activation(
                      aT8_t[:, mb * PW:mb * PW + pw_], ps[:, :pw_], AF.Exp,
                      scale=cc_s[:, CSX:CSX + 1], bias=cc_s[:, CEB:CEB + 1])

                for (g0, gw) in grps:
                  off = g0 - p0
                  # l-row via ones-column block, then R = 1/l broadcast
                  pl = pat.tile([128, 512], f32, tag="at", name="at")
                  for k in range(9):
                    nc.tensor.matmul(
                        pl[:, :gw], vT8v[:, 2 * k:2 * k + 2, 384:512],
                        aT8v[:, 2 * k:2 * k + 2, off:off + gw],
                        start=(k == 0), stop=(k == 8), perf_mode=DR)
                  rrow = stp.tile([1, 512], f32, tag="rrow", name="rrow")
                  nc.vector.reciprocal(rrow[0:1, :gw], pl[0:1, :gw])
                  nc.sync.dma_start(rd_d[0:1, g0:g0 + gw], rrow[0:1, :gw])
                  Rb = rbp.tile([128, 512], f32, tag="Rb", name="Rb")
                  bsrc = bass.AP(rd_d.tensor, rd_d.offset + g0,
                                 [[0, 128], [1, gw]])
                  nc.sync.dma_start(Rb[:, :gw], bsrc)
                  # attn channel blocks: attn8 = (pa * sat) * R
                  for mc in range(NC):
                    pa = pat.tile([128, 512], f32, tag="at", name="at")
                    for k in range(9):
                      nc.tensor.matmul(
                          pa[:, :gw],
                          vT8v[:, 2 * k:2 * k + 2, mc * 128:(mc + 1) * 128],
                          aT8v[:, 2 * k:2 * k + 2, off:off + gw],
                          start=(k == 0), stop=(k == 8), perf_mode=DR)
                    nc.vector.scalar_tensor_tensor(
                        at8v[:, mc, g0:g0 + gw], pa[:, :gw],
                        cc_s[:, CSAT:CSAT + 1], Rb[:, :gw],
                        op0=ALU.mult, op1=ALU.mult)
                  # proj + residual accumulate into x1 (in place)
                  for mc in range(NC):
                    ps = pmm.tile([128, 512], f32, tag="mm", name="mm")
                    for k in range(2):
                      nc.tensor.matmul(
                          ps[:, :gw],
                          pw8v[:, 2 * k:2 * k + 2, mc * 128:(mc + 1) * 128],
                          at8v[:, 2 * k:2 * k + 2, g0:g0 + gw],
                          start=(k == 0), stop=(k == 1), perf_mode=DR)
                    nc.vector.scalar_tensor_tensor(
                        x1_t[mc][:, g0:g0 + gw], ps[:, :gw],
                        cc_s[:, CSP:CSP + 1], x1_t[mc][:, g0:g0 + gw],
                        op0=ALU.mult, op1=ALU.add)
                    nc.gpsimd.tensor_scalar(
                        out=x18v[:, mc, g0:g0 + gw],
                        in0=x1_t[mc][:, g0:g0 + gw],
                        scalar1=cc_s[:, CAX1:CAX1 + 1], scalar2=None,
                        op0=ALU.mult)

            # ---- MLP --------------------------------------------------
            with ExitStack() as mlp_scope:
              hp = mlp_scope.enter_context(tc.tile_pool(name="hp", bufs=1))
              gp = mlp_scope.enter_context(tc.tile_pool(name="gp", bufs=1))
              outp = mlp_scope.enter_context(
                  tc.tile_pool(name="outp", bufs=4))
              pdw = mlp_scope.enter_context(
                  tc.tile_pool(name="pdw", bufs=2, space="PSUM"))

              h8_t = [hp.tile([128, PADN], fp8, tag=f"h{hc}", name=f"h{hc}")
                      for hc in range(NH)]
              for hc in range(NH):
                hv = h8_t[hc][:, :PAD * PAD].rearrange(
                    "p (y x) -> p y x", y=PAD)
                nc.gpsimd.memset(hv[:, 0, :], 0.0)
                nc.gpsimd.memset(hv[:, PAD - 1, :], 0.0)
                nc.gpsimd.memset(hv[:, :, 0], 0.0)
                nc.gpsimd.memset(hv[:, :, PAD - 1], 0.0)
                nc.gpsimd.memset(h8_t[hc][:, PAD * PAD:], 0.0)

              # fc1 for all tiles first (dw of tile ti reads h8 rows from
              # fc1 of tile ti+1, so interleaving would stall the PE)
              for ti, (n0, nn) in enumerate(NT6):
                y0 = ti * RPT
                for hc in range(NH):
                  ps = pmm.tile([128, 512], f32, tag="mm", name="mm")
                  for k in range(2):
                    nc.tensor.matmul(
                        ps[:, :nn],
                        f1w8v[:, 2 * k:2 * k + 2, hc * 128:(hc + 1) * 128],
                        x18v[:, 2 * k:2 * k + 2, n0:n0 + nn],
                        start=(k == 0), stop=(k == 1), perf_mode=DR)
                  hv = h8_t[hc][:, :PAD * PAD].rearrange(
                      "p (y x) -> p y x", y=PAD)
                  dst = hv[:, y0 + 1:y0 + 1 + RPT, 1:1 + W]
                  psv = ps[:, :nn].rearrange("p (y x) -> p y x", y=RPT)
                  if hc % 5 < 3:
                    nc.vector.tensor_scalar(
                        out=dst, in0=psv,
                        scalar1=cc_s[:, CSH:CSH + 1],
                        scalar2=cc_s[:, CHB + hc:CHB + hc + 1],
                        op0=ALU.mult, op1=ALU.add)
                  else:
                    nc.scalar.activation(
                        dst, psv, AF.Identity,
                        scale=cc_s[:, CSH:CSH + 1],
                        bias=cc_s[:, CHB + hc:CHB + hc + 1])

              # dw (5 DR tap pairs) + gelu -> g8, per tile
              g8_t = [gp.tile([128, NH * 384], fp8, tag=f"g8_{ti}",
                              name=f"g8_{ti}") for ti in range(6)]
              for ti, (n0, nn) in enumerate(NT6):
                y0 = ti * RPT
                g8v = g8_t[ti][:].rearrange("p (k f) -> p k f", k=NH)
                for hc in range(NH):
                  hh = h8_t[hc][:]
                  pd = pdw.tile([128, 512], f32, tag="dw", name="dw")
                  pdv = pd[:, :nn].rearrange("p (y x) -> p y x", y=RPT)
                  dd = dwd8_s[:]
                  for pi, (d0, dstp) in enumerate(DWPAIRS):
                    lhsT = bass.AP(dd.tensor,
                                   dd.offset + (hc * 10 + 2 * pi) * 128,
                                   [list(dd.ap[0]), [128, 2], [1, 128]])
                    rhs = bass.AP(hh.tensor, hh.offset + y0 * PAD + d0,
                                  [list(hh.ap[0]), [dstp, 2], [PAD, RPT],
                                   [1, W]])
                    nc.tensor.matmul(pdv, lhsT, rhs, start=(pi == 0),
                                     stop=(pi == 4), perf_mode=DR)
                  nc.scalar.activation(
                      g8_t[ti][:, hc * 384:hc * 384 + nn], pd[:, :nn],
                      AF.Gelu, scale=cc_s[:, CSG:CSG + 1],
                      bias=cc_s[:, CDWB + hc:CDWB + hc + 1])

              # fc2 + residual, mc-major so next iteration's x loads can
              # begin as soon as an output channel block is complete
              for mc in range(NC):
                for ti, (n0, nn) in enumerate(NT6):
                  g8v = g8_t[ti][:].rearrange("p (k f) -> p k f", k=NH)
                  ps = pmm.tile([128, 512], f32, tag="mm", name="mm")
                  for k in range(6):
                    nc.tensor.matmul(
                        ps[:, :nn],
                        f2w8v[:, 2 * k:2 * k + 2, mc * 128:(mc + 1) * 128],
                        g8v[:, 2 * k:2 * k + 2, :nn],
                        start=(k == 0), stop=(k == 5), perf_mode=DR)
                  ot = outp.tile([128, 384], f32, tag="ot", name="ot")
                  nc.vector.scalar_tensor_tensor(
                      ot[:, :nn], ps[:, :nn], cc_s[:, CSF2:CSF2 + 1],
                      x1_t[mc][:, n0:n0 + nn], op0=ALU.mult, op1=ALU.add)
                  nc.sync.dma_start(
                      out_d[mc * 128:(mc + 1) * 128, n0:n0 + nn],
                      ot[:, :nn])

    nc.compile()
    return nc


def _chunks(v, k):
    return np.ascontiguousarray(np.asarray(v, np.float32).reshape(k, 128).T)


def _lhsT8(Wm, scale, kin):
    """W [out, in] (y = W x) -> DoubleRow lhsT fp8 [128, kin*out]."""
    out_, in_ = Wm.shape
    T = np.zeros((kin * 128, out_), np.float32)
    T[:in_] = Wm.T * scale
    T = T.reshape(kin, 128, out_).transpose(1, 0, 2).reshape(128, kin * out_)
    return np.ascontiguousarray(T).astype(FP8)


def _fold_inputs(inputs):
    """Host-side weight folding + fp8 quantization. Returns (w map, xs)."""
    f = np.float32
    g = {k: np.asarray(v, f) for k, v in inputs.items()}
    s1 = g['bn1_g'] / np.sqrt(g['bn1_v'] + EPS)
    t1 = g['bn1_b'] - g['bn1_m'] * s1
    qw = g['q_w'] * s1[None, :]; qb = g['q_w'] @ t1 + g['q_b']
    kw = g['k_w'] * s1[None, :]
    vw = g['v_w'] * s1[None, :]; vb = g['v_w'] @ t1 + g['v_b']
    ls1, ls2 = g['ls1'], g['ls2']
    pw = ls1[:, None] * g['po_w']
    alpha1 = 1.0 + ls1 * s1
    beta1 = ls1 * (g['po_b'] + t1) + pw @ vb
    s2 = g['bn2_g'] / np.sqrt(g['bn2_v'] + EPS)
    t2 = g['bn2_b'] - g['bn2_m'] * s2
    f1w = g['fc1_w'] * s2[None, :]
    f1b = g['fc1_w'] @ t2 + g['fc1_b']
    f2w = ls2[:, None] * g['fc2_w']
    f2b = ls2 * g['fc2_b']
    x1bias = beta1 + f2b          # carried by x1
    f1bc = f1b - f1w @ f2b        # compensate f2b inside x1
    rC = 1.0 / np.sqrt(f(C))
    A = (kw.T @ qw) * rC
    w2 = (kw.T @ qb) * rC

    x = g['x']
    sdx = float(x.std()) + 1e-9
    ax = FMAX / (float(np.abs(x).max()) + 1e-9)
    aA = FMAX / (float(np.abs(A).max()) + 1e-12)
    umax = 8.0 * float(np.linalg.norm(A, axis=0).max()) * sdx \
        + float(np.abs(w2).max()) + 1e-9
    au = FMAX / umax
    avw = FMAX / (float(np.abs(vw).max()) + 1e-12)
    vmax = 8.0 * float(np.linalg.norm(vw, axis=1).max()) * sdx + 1e-9
    av = FMAX / vmax
    aat = av
    apw = FMAX / (float(np.abs(pw).max()) + 1e-12)
    ax1 = FMAX / (float(np.abs(x).max()) * 1.25
                  + float(np.abs(x1bias).max()) + 1e-9)
    af1 = FMAX / (float(np.abs(f1w).max()) + 1e-12)
    hmax = 8.0 * float(np.linalg.norm(f1w, axis=1).max()) * (1.2 * sdx) \
        + float(np.abs(f1bc).max()) + 1e-9
    ah = FMAX / hmax
    adw = FMAX / (float(np.abs(g['dw_w']).max()) + 1e-12)
    af2 = FMAX / (float(np.abs(f2w).max()) + 1e-12)

    dww = g['dw_w'].reshape(HID, 9)
    dd = np.zeros((128, NH, 10, 128), f)
    ar = np.arange(128)
    for hc in range(NH):
        for tap in range(9):
            dd[ar, hc, tap, ar] = dww[hc * 128 + ar, tap] * adw

    cc = np.zeros((128, 48), f)
    cc[:, 0] = ax
    cc[:, 1] = au / (aA * ax)
    cc[:, 2:5] = _chunks(au * w2, NC)
    cc[:, 5] = 1.0 / (au * ax)
    cc[:, 6] = EXPB
    cc[:, 7] = av / (ax * avw)
    cc[:, 8] = aat / av
    cc[:, 9] = 1.0 / (apw * aat)
    cc[:, 10:13] = _chunks(x1bias, NC)
    cc[:, 13:16] = _chunks(alpha1, NC)
    cc[:, 16] = ax1
    cc[:, 17] = ah / (af1 * ax1)
    cc[:, 18:30] = _chunks(ah * f1bc, NH)
    cc[:, 30] = 1.0 / (adw * ah)
    cc[:, 31:43] = _chunks(g['dw_b'], NH)
    cc[:, 43] = 1.0 / af2

    w = {
        'A8': _lhsT8(A, aA, NCP),
        'vw8': _lhsT8(vw, avw, NCP),
        'pw8': _lhsT8(pw, apw, NCP),
        'f1w8': _lhsT8(f1w, af1, NCP),
        'f2w8': _lhsT8(f2w, af2, NH),
        'dwd8': np.ascontiguousarray(
            dd.reshape(128, NH * 10 * 128)).astype(FP8),
        'cc': np.ascontiguousarray(cc),
    }
    xs = [np.ascontiguousarray(g['x'][b].reshape(C, N))
          for b in range(g['x'].shape[0])]
    return w, xs


def get_program():
    global _PROG
    if _PROG is None:
        _PROG = _build_program()
    return _PROG


def kernel(**inputs):
    from concourse.bass_utils import run_bass_kernel_spmd
    nc = get_program()
    w, xs = _fold_inputs(inputs)
    B = len(xs)
    in_maps = [{**w, 'x': xs[b]} for b in range(B)]
    res = run_bass_kernel_spmd(nc, in_maps, list(range(B)))
    out = np.stack([res.results[b]['out'].reshape(C, H, W) for b in range(B)])
    return out.astype(inputs['x'].dtype if hasattr(inputs['x'], 'dtype')
                      else np.float32)
